# revision 6
# baseline (speedup 1.0000x reference)
"""AxisMoE (top-2 of 8 experts) TRN2 kernel.

Strategy: data-parallel over batch (B=8 -> 8 NeuronCores, one batch row each).
Per core, dense-expert compute with on-device gating:
  - gate logits via PE matmul (h stationary, gate weights moving), softmax /
    top-2 / combine weights / entropy on DVE+ACT,
  - expert matmuls in float32r (fp22 mantissa truncation, 4x faster than fp32)
    with h-tile stationary [di,t] and expert weights moving [di,do],
    PSUM-accumulated over di,
  - per-expert drain fused on DVE: acc = psum * combine[:,e] + acc,
  - expert biases applied via a single K=8 matmul: combine.T @ exp_b.

Host side only reshapes/transposes inputs into DMA-friendly layouts and
distributes/collects per-core arrays.
"""

import numpy as np

B, S, D, DA, E = 8, 4096, 1024, 128, 8
N_CORES = 8
TT = 128            # tokens per t-tile (PE stationary M)
NT_FULL = S // TT   # 32 t-tiles per core
NHF = 2             # d_out halves (512 each)
HFW = D // NHF      # 512

_BUILD_CACHE = {}


def _build(n_ttiles=NT_FULL, n_repeat=1):
    """Build + compile the Bass program (same SPMD program for all cores)."""
    import concourse.bacc as bacc
    import concourse.mybir as mybir
    import concourse.tile as tile
    from concourse.masks import make_identity

    f32 = mybir.dt.float32
    f32r = mybir.dt.float32r
    ALU = mybir.AluOpType
    ACT = mybir.ActivationFunctionType
    AX = mybir.AxisListType

    nt = n_ttiles
    ntok = nt * TT

    nc = bacc.Bacc("TRN2", target_bir_lowering=False, debug=False,
                   enable_asserts=True, num_devices=N_CORES)

    hx = nc.dram_tensor("hx", [nt, 128, 8, TT], f32, kind="ExternalInput").ap()
    wx = nc.dram_tensor("wx", [NHF, E, 128, 8, HFW], f32r, kind="ExternalInput").ap()
    gw = nc.dram_tensor("gw", [128, 9, E], f32, kind="ExternalInput").ap()
    av = nc.dram_tensor("av", [128, 1], f32, kind="ExternalInput").ap()
    gb = nc.dram_tensor("gb", [1, E], f32, kind="ExternalInput").ap()
    eb = nc.dram_tensor("eb", [E, D], f32, kind="ExternalInput").ap()
    out = nc.dram_tensor("out", [ntok, D], f32, kind="ExternalOutput").ap()
    ent = nc.dram_tensor("ent", [1, 1], f32, kind="ExternalOutput").ap()

    with tile.TileContext(nc) as tc:
        with (
            tc.tile_pool(name="const", bufs=1) as cpool,
            tc.tile_pool(name="w", bufs=8) as wpool,
            tc.tile_pool(name="h", bufs=3) as hpool,
            tc.tile_pool(name="acc", bufs=3) as accpool,
            tc.tile_pool(name="gate", bufs=2) as gpool,
            tc.tile_pool(name="ps", bufs=3, space="PSUM") as pspool,
            tc.tile_pool(name="psz", bufs=2, space="PSUM") as zpool,
            tc.tile_pool(name="psct", bufs=1, space="PSUM") as ctpool,
            tc.tile_pool(name="psb", bufs=2, space="PSUM") as bpool,
        ):
            def body():
                # ---- constants / setup ----
                gw_sb = cpool.tile([128, 9, E], f32, tag="gw")
                nc.sync.dma_start(gw_sb[:], gw[:])
                av_sb = cpool.tile([128, 1], f32, tag="av")
                nc.sync.dma_start(av_sb[:], av[:])
                gb_sb = cpool.tile([1, E], f32, tag="gb")
                nc.sync.dma_start(gb_sb[:], gb[:])
                eb_sb = cpool.tile([E, D], f32, tag="eb")
                nc.sync.dma_start(eb_sb[:], eb[:])
                ident = cpool.tile([128, 128], f32, tag="ident")
                make_identity(nc, ident[:])
                ones_col = cpool.tile([128, 1], f32, tag="ones")
                nc.vector.memset(ones_col[:], 1.0)

                # za[e] = a . gate_W[:, D:D+DA].T + gate_b  (broadcast to 128 rows)
                za_ps = zpool.tile([1, E], f32, tag="zsmall")
                nc.tensor.matmul(za_ps[:], av_sb[:], gw_sb[:, 8, :],
                                 start=True, stop=True)
                za_sb = cpool.tile([1, E], f32, tag="za")
                nc.vector.tensor_tensor(za_sb[:], za_ps[:], gb_sb[:], op=ALU.add)
                za_bc = cpool.tile([128, E], f32, tag="zabc")
                nc.gpsimd.partition_broadcast(za_bc[:], za_sb[:])

                comb_all = cpool.tile([128, nt, E], f32, tag="comb")
                combT_all = cpool.tile([E, nt * TT], f32, tag="combT")
                entc = cpool.tile([128, nt], f32, tag="entc")

                for hf in range(NHF):
                    # resident expert-weight tiles for this d_out half
                    w_tiles = []
                    for e in range(E):
                        wt = wpool.tile([128, 8, HFW], f32r, tag="w")
                        nc.sync.dma_start(wt[:], wx[hf, e])
                        w_tiles.append(wt)

                    for tt in range(nt):
                        ht = hpool.tile([128, 8, TT], f32, tag="h")
                        nc.sync.dma_start(ht[:], hx[tt])
                        # fp22-rounded copy for the expert matmuls (gate keeps
                        # full fp32 h so top-2 selection stays exact)
                        htr = hpool.tile([128, 8, TT], f32r, tag="hr")
                        nc.scalar.activation(htr[:], ht[:], ACT.Copy)

                        if hf == 0:
                            # ---- gate for this token tile ----
                            zp = zpool.tile([128, E], f32, tag="zsmall")
                            for dt in range(8):
                                nc.tensor.matmul(zp[:], ht[:, dt, :],
                                                 gw_sb[:, dt, :],
                                                 start=(dt == 0), stop=(dt == 7))
                            z = gpool.tile([128, E], f32, tag="z")
                            nc.vector.tensor_tensor(z[:], zp[:], za_bc[:], op=ALU.add)
                            m1 = gpool.tile([128, 1], f32, tag="m1")
                            nc.vector.tensor_reduce(m1[:], z[:], axis=AX.X, op=ALU.max)
                            nm1 = gpool.tile([128, 1], f32, tag="nm1")
                            nc.vector.tensor_scalar(nm1[:], m1[:], -1.0, None, op0=ALU.mult)
                            p = gpool.tile([128, E], f32, tag="p")
                            nc.scalar.activation(p[:], z[:], ACT.Exp, bias=nm1[:], scale=1.0)
                            s = gpool.tile([128, 1], f32, tag="s")
                            nc.vector.tensor_reduce(s[:], p[:], axis=AX.X, op=ALU.add)
                            # entropy: sum g*ln g = (sum p*z)/s - m1 - ln s
                            pz0 = gpool.tile([128, E], f32, tag="pz0")
                            nc.vector.tensor_tensor(pz0[:], p[:], z[:], op=ALU.mult)
                            pz = gpool.tile([128, 1], f32, tag="pz")
                            nc.vector.tensor_reduce(pz[:], pz0[:], axis=AX.X, op=ALU.add)
                            ls = gpool.tile([128, 1], f32, tag="ls")
                            nc.scalar.activation(ls[:], s[:], ACT.Ln)
                            rs = gpool.tile([128, 1], f32, tag="rs")
                            nc.vector.reciprocal(rs[:], s[:])
                            t1 = gpool.tile([128, 1], f32, tag="t1")
                            nc.vector.tensor_tensor(t1[:], pz[:], rs[:], op=ALU.mult)
                            t2 = gpool.tile([128, 1], f32, tag="t2")
                            nc.vector.tensor_tensor(t2[:], t1[:], nm1[:], op=ALU.add)
                            nc.vector.tensor_tensor(entc[:, tt:tt + 1], t2[:], ls[:],
                                                    op=ALU.subtract)
                            # top-2 mask + normalized combine weights
                            eq = gpool.tile([128, E], f32, tag="eq")
                            nc.vector.tensor_scalar(eq[:], z[:], m1[:], None, op0=ALU.is_ge)
                            zm = gpool.tile([128, E], f32, tag="zm")
                            nc.vector.scalar_tensor_tensor(zm[:], eq[:], -1e30, z[:],
                                                           op0=ALU.mult, op1=ALU.add)
                            m2 = gpool.tile([128, 1], f32, tag="m2")
                            nc.vector.tensor_reduce(m2[:], zm[:], axis=AX.X, op=ALU.max)
                            sel = gpool.tile([128, E], f32, tag="sel")
                            nc.vector.tensor_scalar(sel[:], z[:], m2[:], None, op0=ALU.is_ge)
                            pm = gpool.tile([128, E], f32, tag="pm")
                            nc.vector.tensor_tensor(pm[:], p[:], sel[:], op=ALU.mult)
                            d2 = gpool.tile([128, 1], f32, tag="d2")
                            nc.vector.tensor_reduce(d2[:], pm[:], axis=AX.X, op=ALU.add)
                            rd2 = gpool.tile([128, 1], f32, tag="rd2")
                            nc.vector.reciprocal(rd2[:], d2[:])
                            nc.vector.tensor_scalar(comb_all[:, tt, :], pm[:], rd2[:],
                                                    None, op0=ALU.mult)
                            # combine.T for the bias matmul
                            ctp = ctpool.tile([E, TT], f32, tag="ct")
                            nc.tensor.transpose(ctp[:], comb_all[:, tt, :], ident[:])
                            nc.vector.tensor_copy(
                                combT_all[:, tt * TT:(tt + 1) * TT], ctp[:])

                        # ---- experts ----
                        acc = accpool.tile([128, HFW], f32, tag="acc")
                        for e in range(E):
                            ps = pspool.tile([128, HFW], f32, tag="ps")
                            for dt in range(8):
                                nc.tensor.matmul(
                                    ps[:],
                                    htr[:, dt, :],
                                    w_tiles[e][:, dt, :],
                                    start=(dt == 0), stop=(dt == 7))
                            cs = comb_all[:, tt, e:e + 1]
                            if e == 0:
                                nc.vector.tensor_scalar(acc[:], ps[:], cs, None,
                                                        op0=ALU.mult)
                            else:
                                nc.vector.scalar_tensor_tensor(acc[:], ps[:], cs, acc[:],
                                                               op0=ALU.mult, op1=ALU.add)
                        # expert-bias term: combine @ exp_b  (K=8 matmul)
                        bp = bpool.tile([128, HFW], f32, tag="bias")
                        nc.tensor.matmul(bp[:],
                                         combT_all[:, tt * TT:(tt + 1) * TT],
                                         eb_sb[:, hf * HFW:(hf + 1) * HFW],
                                         start=True, stop=True)
                        nc.vector.tensor_tensor(acc[:], acc[:], bp[:], op=ALU.add)
                        nc.sync.dma_start(
                            out[tt * TT:(tt + 1) * TT, hf * HFW:(hf + 1) * HFW], acc[:])

                # ---- entropy reduction ----
                er = gpool.tile([128, 1], f32, tag="er")
                nc.vector.tensor_reduce(er[:], entc[:], axis=AX.X, op=ALU.add)
                ep = zpool.tile([1, 1], f32, tag="zsmall")
                nc.tensor.matmul(ep[:], er[:], ones_col[:], start=True, stop=True)
                esb = gpool.tile([1, 1], f32, tag="esb")
                nc.vector.tensor_copy(esb[:], ep[:])
                nc.sync.dma_start(ent[:], esb[:])

            if n_repeat > 1:
                with tc.For_i(0, n_repeat, 1):
                    body()
            else:
                body()

    nc.compile()
    return nc


def _round_fp22(x):
    """Round fp32 to fp22 (13 mantissa bits) — matches PE float32r ingest."""
    v = x.view(np.uint32)
    v = (v + np.uint32(0x200)) & np.uint32(0xFFFFFC00)
    return v.view(np.float32)


def _prep_shared(gate_W, gate_b, exp_W, exp_b):
    """Host-side layout prep shared across cores."""
    # wx[hf, e, p, dt, f] = exp_W[e, hf*512+f, dt*128+p], pre-rounded to fp22
    wx = _round_fp22(np.ascontiguousarray(
        exp_W.reshape(E, NHF, HFW, 8, 128).transpose(1, 0, 4, 3, 2)))
    # gw[p, dt, e] = gate_W[e, dt*128+p]
    gwl = np.ascontiguousarray(gate_W.reshape(E, 9, 128).transpose(2, 1, 0))
    gbl = np.ascontiguousarray(gate_b.reshape(1, E))
    ebl = np.ascontiguousarray(exp_b)
    return wx, gwl, gbl, ebl


def _prep_core(h_b, a_b, n_ttiles=NT_FULL):
    # hx[tt, p, dt, f] = h[tt*128+f, dt*128+p]
    ntok = n_ttiles * TT
    hx = np.ascontiguousarray(
        h_b[:ntok].reshape(n_ttiles, TT, 8, 128).transpose(0, 3, 2, 1))
    avl = np.ascontiguousarray(a_b.reshape(128, 1))
    return hx, avl


def kernel(h, a, gate_W, gate_b, exp_W, exp_b):
    from concourse import bass_utils

    h = np.asarray(h, dtype=np.float32)
    a = np.asarray(a, dtype=np.float32)
    gate_W = np.asarray(gate_W, dtype=np.float32)
    gate_b = np.asarray(gate_b, dtype=np.float32)
    exp_W = np.asarray(exp_W, dtype=np.float32)
    exp_b = np.asarray(exp_b, dtype=np.float32)

    key = (NT_FULL, 1)
    if key not in _BUILD_CACHE:
        _BUILD_CACHE[key] = _build(*key)
    nc = _BUILD_CACHE[key]

    wx, gwl, gbl, ebl = _prep_shared(gate_W, gate_b, exp_W, exp_b)
    in_maps = []
    for b in range(N_CORES):
        hx, avl = _prep_core(h[b], a[b])
        in_maps.append({"hx": hx, "wx": wx, "gw": gwl, "av": avl,
                        "gb": gbl, "eb": ebl})

    res = bass_utils.run_bass_kernel_spmd(nc, in_maps, core_ids=list(range(N_CORES)))

    out = np.stack([res.results[b]["out"] for b in range(N_CORES)], axis=0)
    ent_sum = np.float64(0.0)
    for b in range(N_CORES):
        ent_sum += np.float64(res.results[b]["ent"][0, 0])
    entropy_loss = np.float32(-(ent_sum / (B * S)))
    stability_loss = np.float32(0.0)
    return out, entropy_loss, stability_loss


# revision 31
# speedup vs baseline: 1.0676x; 1.0676x over previous
"""AxisMoE (top-2 of 8 experts) TRN2 kernel.

Strategy: data-parallel over batch (B=8 -> 8 NeuronCores, one batch row each).
Per core, dense-expert compute with on-device gating:
  - gate logits via PE matmul (h stationary, gate weights moving), softmax /
    top-2 / combine weights / entropy on DVE+ACT,
  - expert matmuls in float32r (fp22 mantissa truncation, 4x faster than fp32)
    with h-tile stationary [di,t] and expert weights moving [di,do],
    PSUM-accumulated over di,
  - per-expert drain fused on DVE: acc = psum * combine[:,e] + acc,
  - expert biases applied via a single K=8 matmul: combine.T @ exp_b.

Host side only reshapes/transposes inputs into DMA-friendly layouts and
distributes/collects per-core arrays.
"""

import numpy as np

B, S, D, DA, E = 8, 4096, 1024, 128, 8
N_CORES = 8
TT = 128            # tokens per t-tile (PE stationary M)
NT_FULL = S // TT   # 32 t-tiles per core
NHF = 2             # d_out halves (512 each)
HFW = D // NHF      # 512

_BUILD_CACHE = {}


def _build(n_ttiles=NT_FULL, n_repeat=1, mm_order="e_outer", bias_f32r=True,
           gate_prephase=False, h_bufs=3, acc_bufs=3,
           skip_drain=False, skip_gate=False, skip_bias=False,
           skip_outdma=False, skip_wdma=False, skip_hconv=False,
           skip_mm=False):
    """Build + compile the Bass program (same SPMD program for all cores)."""
    import concourse.bacc as bacc
    import concourse.mybir as mybir
    import concourse.tile as tile
    from concourse.masks import make_identity

    f32 = mybir.dt.float32
    f32r = mybir.dt.float32r
    ALU = mybir.AluOpType
    ACT = mybir.ActivationFunctionType
    AX = mybir.AxisListType

    nt = n_ttiles
    ntok = nt * TT

    nc = bacc.Bacc("TRN2", target_bir_lowering=False, debug=False,
                   enable_asserts=True, num_devices=N_CORES)

    hx = nc.dram_tensor("hx", [nt, 128, 8, TT], f32, kind="ExternalInput").ap()
    wx = nc.dram_tensor("wx", [NHF, E, 128, 8, HFW], f32r, kind="ExternalInput").ap()
    gw = nc.dram_tensor("gw", [128, 9, E], f32, kind="ExternalInput").ap()
    av = nc.dram_tensor("av", [128, 1], f32, kind="ExternalInput").ap()
    gb = nc.dram_tensor("gb", [1, E], f32, kind="ExternalInput").ap()
    eb = nc.dram_tensor("eb", [E, D], f32r if bias_f32r else f32,
                        kind="ExternalInput").ap()
    out = nc.dram_tensor("out", [ntok, D], f32, kind="ExternalOutput").ap()
    ent = nc.dram_tensor("ent", [1, 1], f32, kind="ExternalOutput").ap()

    if skip_mm:
        skip_drain = True
    if skip_gate:
        skip_bias = True  # bias matmul needs combT from the gate phase
    dt_outer = mm_order == "dt_outer"
    round4 = mm_order == "round4"
    from contextlib import ExitStack
    with tile.TileContext(nc) as tc:
        with ExitStack() as ctx:
            cpool = ctx.enter_context(tc.tile_pool(name="const", bufs=1))
            wpool = ctx.enter_context(tc.tile_pool(name="w", bufs=8))
            hpool = ctx.enter_context(tc.tile_pool(name="h", bufs=h_bufs))
            accpool = ctx.enter_context(tc.tile_pool(name="acc", bufs=acc_bufs))
            gpool = ctx.enter_context(tc.tile_pool(name="gate", bufs=2))
            ps_bufs = 8 if dt_outer else (4 if round4 else 3)
            pspool = ctx.enter_context(
                tc.tile_pool(name="ps", bufs=ps_bufs, space="PSUM"))
            if dt_outer:
                # everything shares the single 8-bank pool
                zpool = ctpool = bpool = pspool
                ztag = cttag = btag = "ps"
            else:
                zpool = ctx.enter_context(
                    tc.tile_pool(name="psz", bufs=2, space="PSUM"))
                ctpool = ctx.enter_context(
                    tc.tile_pool(name="psct", bufs=1, space="PSUM"))
                bpool = ctx.enter_context(
                    tc.tile_pool(name="psb", bufs=1 if round4 else 2, space="PSUM"))
                ztag, cttag, btag = "zsmall", "ct", "bias"
            def body():
                # ---- constants / setup ----
                gw_sb = cpool.tile([128, 9, E], f32, tag="gw")
                nc.sync.dma_start(gw_sb[:], gw[:])
                av_sb = cpool.tile([128, 1], f32, tag="av")
                nc.sync.dma_start(av_sb[:], av[:])
                gb_sb = cpool.tile([1, E], f32, tag="gb")
                nc.sync.dma_start(gb_sb[:], gb[:])
                eb_sb = cpool.tile([E, D], f32r if bias_f32r else f32, tag="eb")
                nc.sync.dma_start(eb_sb[:], eb[:])
                ident = cpool.tile([128, 128], f32, tag="ident")
                make_identity(nc, ident[:])
                ones_col = cpool.tile([128, 1], f32, tag="ones")
                nc.vector.memset(ones_col[:], 1.0)

                # za[e] = a . gate_W[:, D:D+DA].T + gate_b  (broadcast to 128 rows)
                za_ps = zpool.tile([1, E], f32, tag=ztag)
                nc.tensor.matmul(za_ps[:], av_sb[:], gw_sb[:, 8, :],
                                 start=True, stop=True)
                za_sb = cpool.tile([1, E], f32, tag="za")
                nc.vector.tensor_tensor(za_sb[:], za_ps[:], gb_sb[:], op=ALU.add)
                za_bc = cpool.tile([128, E], f32, tag="zabc")
                nc.gpsimd.partition_broadcast(za_bc[:], za_sb[:])

                comb_all = cpool.tile([128, nt, E], f32, tag="comb")
                combT_all = cpool.tile([E, nt * TT], f32r if bias_f32r else f32,
                                       tag="combT")
                entc = cpool.tile([128, nt], f32, tag="entc")
                if skip_gate and not skip_drain:  # timing-ablation only
                    nc.vector.memset(comb_all[:], 0.125)

                def gate_tile(tt, ht):
                    if True:
                        if True:
                            # ---- gate for this token tile ----
                            zp = zpool.tile([128, E], f32, tag=ztag)
                            for dt in range(8):
                                nc.tensor.matmul(zp[:], ht[:, dt, :],
                                                 gw_sb[:, dt, :],
                                                 start=(dt == 0), stop=(dt == 7))
                            z = gpool.tile([128, E], f32, tag="z")
                            nc.vector.tensor_tensor(z[:], zp[:], za_bc[:], op=ALU.add)
                            m1 = gpool.tile([128, 1], f32, tag="m1")
                            nc.vector.tensor_reduce(m1[:], z[:], axis=AX.X, op=ALU.max)
                            nm1 = gpool.tile([128, 1], f32, tag="nm1")
                            nc.vector.tensor_scalar(nm1[:], m1[:], -1.0, None, op0=ALU.mult)
                            p = gpool.tile([128, E], f32, tag="p")
                            nc.scalar.activation(p[:], z[:], ACT.Exp, bias=nm1[:], scale=1.0)
                            s = gpool.tile([128, 1], f32, tag="s")
                            nc.vector.tensor_reduce(s[:], p[:], axis=AX.X, op=ALU.add)
                            # entropy: sum g*ln g = (sum p*z)/s - m1 - ln s
                            pz0 = gpool.tile([128, E], f32, tag="pz0")
                            nc.vector.tensor_tensor(pz0[:], p[:], z[:], op=ALU.mult)
                            pz = gpool.tile([128, 1], f32, tag="pz")
                            nc.vector.tensor_reduce(pz[:], pz0[:], axis=AX.X, op=ALU.add)
                            ls = gpool.tile([128, 1], f32, tag="ls")
                            nc.scalar.activation(ls[:], s[:], ACT.Ln)
                            rs = gpool.tile([128, 1], f32, tag="rs")
                            nc.vector.reciprocal(rs[:], s[:])
                            t1 = gpool.tile([128, 1], f32, tag="t1")
                            nc.vector.tensor_tensor(t1[:], pz[:], rs[:], op=ALU.mult)
                            t2 = gpool.tile([128, 1], f32, tag="t2")
                            nc.vector.tensor_tensor(t2[:], t1[:], nm1[:], op=ALU.add)
                            nc.vector.tensor_tensor(entc[:, tt:tt + 1], t2[:], ls[:],
                                                    op=ALU.subtract)
                            # top-2 mask + normalized combine weights
                            eq = gpool.tile([128, E], f32, tag="eq")
                            nc.vector.tensor_scalar(eq[:], z[:], m1[:], None, op0=ALU.is_ge)
                            zm = gpool.tile([128, E], f32, tag="zm")
                            nc.vector.scalar_tensor_tensor(zm[:], eq[:], -1e30, z[:],
                                                           op0=ALU.mult, op1=ALU.add)
                            m2 = gpool.tile([128, 1], f32, tag="m2")
                            nc.vector.tensor_reduce(m2[:], zm[:], axis=AX.X, op=ALU.max)
                            sel = gpool.tile([128, E], f32, tag="sel")
                            nc.vector.tensor_scalar(sel[:], z[:], m2[:], None, op0=ALU.is_ge)
                            pm = gpool.tile([128, E], f32, tag="pm")
                            nc.vector.tensor_tensor(pm[:], p[:], sel[:], op=ALU.mult)
                            d2 = gpool.tile([128, 1], f32, tag="d2")
                            nc.vector.tensor_reduce(d2[:], pm[:], axis=AX.X, op=ALU.add)
                            rd2 = gpool.tile([128, 1], f32, tag="rd2")
                            nc.vector.reciprocal(rd2[:], d2[:])
                            nc.vector.tensor_scalar(comb_all[:, tt, :], pm[:], rd2[:],
                                                    None, op0=ALU.mult)
                            # combine.T for the bias matmul
                            ctp = ctpool.tile([E, TT], f32, tag=cttag)
                            nc.tensor.transpose(ctp[:], comb_all[:, tt, :], ident[:])
                            nc.vector.tensor_copy(
                                combT_all[:, tt * TT:(tt + 1) * TT], ctp[:])

                # ---- gate prephase (overlaps initial weight DMA) ----
                if gate_prephase and not skip_gate:
                    for tt in range(nt):
                        htg = hpool.tile([128, 8, TT], f32, tag="h", name="htg")
                        nc.sync.dma_start(htg[:], hx[tt])
                        gate_tile(tt, htg)

                # ---- expert phases: one pass per d_out half ----
                for hf in range(NHF):
                    # resident expert-weight tiles for this d_out half
                    w_tiles = []
                    for e in range(E):
                        wt = wpool.tile([128, 8, HFW], f32r, tag="w")
                        if not skip_wdma:
                            nc.sync.dma_start(wt[:], wx[hf, e])
                        w_tiles.append(wt)

                    for tt in range(nt):
                        ht = hpool.tile([128, 8, TT], f32, tag="h", name="hte")
                        nc.sync.dma_start(ht[:], hx[tt])
                        # fp22-rounded copy for the expert matmuls (gate keeps
                        # full fp32 h so top-2 selection stays exact)
                        htr = hpool.tile([128, 8, TT], f32r, tag="hr")
                        if not skip_hconv:
                            nc.scalar.activation(htr[:], ht[:], ACT.Copy)
                        if hf == 0 and not gate_prephase and not skip_gate:
                            gate_tile(tt, ht)

                        # ---- experts ----
                        acc = (None if skip_drain else
                               accpool.tile([128, HFW], f32, tag="acc", name="acc"))
                        if dt_outer:
                            ps_tiles = ([] if skip_mm else
                                        [pspool.tile([128, HFW], f32, tag="ps",
                                                     name=f"pse{e}")
                                         for e in range(E)])
                            if not skip_mm:
                                for dt in range(8):
                                    for e in range(E):
                                        nc.tensor.matmul(
                                            ps_tiles[e][:],
                                            htr[:, dt, :],
                                            w_tiles[e][:, dt, :],
                                            start=(dt == 0), stop=(dt == 7))
                            if not skip_drain:
                                for e in range(E):
                                    cs = comb_all[:, tt, e:e + 1]
                                    if e == 0:
                                        nc.vector.tensor_scalar(
                                            acc[:], ps_tiles[0][:], cs, None,
                                            op0=ALU.mult)
                                    else:
                                        nc.vector.scalar_tensor_tensor(
                                            acc[:], ps_tiles[e][:], cs, acc[:],
                                            op0=ALU.mult, op1=ALU.add)
                        elif round4:
                            for rnd in range(2):
                                ps_tiles = [pspool.tile([128, HFW], f32, tag="ps",
                                                        name=f"psr{e}")
                                            for e in range(4)]
                                for dt in range(8):
                                    for i, e in enumerate(range(rnd * 4, rnd * 4 + 4)):
                                        nc.tensor.matmul(
                                            ps_tiles[i][:],
                                            htr[:, dt, :],
                                            w_tiles[e][:, dt, :],
                                            start=(dt == 0), stop=(dt == 7))
                                if not skip_drain:
                                    for i, e in enumerate(range(rnd * 4, rnd * 4 + 4)):
                                        cs = comb_all[:, tt, e:e + 1]
                                        if e == 0:
                                            nc.vector.tensor_scalar(
                                                acc[:], ps_tiles[i][:], cs, None,
                                                op0=ALU.mult)
                                        else:
                                            nc.vector.scalar_tensor_tensor(
                                                acc[:], ps_tiles[i][:], cs, acc[:],
                                                op0=ALU.mult, op1=ALU.add)
                        else:
                            for e in range(E):
                                if skip_mm:
                                    break
                                ps = pspool.tile([128, HFW], f32, tag="ps")
                                if not skip_mm:
                                    for dt in range(8):
                                        nc.tensor.matmul(
                                            ps[:],
                                            htr[:, dt, :],
                                            w_tiles[e][:, dt, :],
                                            start=(dt == 0), stop=(dt == 7))
                                if not skip_drain:
                                    cs = comb_all[:, tt, e:e + 1]
                                    if e == 0:
                                        nc.vector.tensor_scalar(acc[:], ps[:], cs,
                                                                None, op0=ALU.mult)
                                    else:
                                        nc.vector.scalar_tensor_tensor(
                                            acc[:], ps[:], cs, acc[:],
                                            op0=ALU.mult, op1=ALU.add)
                        # expert-bias term: combine @ exp_b  (K=8 matmul)
                        if not skip_bias and not skip_drain:
                            bp = bpool.tile([128, HFW], f32, tag=btag)
                            nc.tensor.matmul(bp[:],
                                             combT_all[:, tt * TT:(tt + 1) * TT],
                                             eb_sb[:, hf * HFW:(hf + 1) * HFW],
                                             start=True, stop=True)
                            nc.vector.tensor_tensor(acc[:], acc[:], bp[:], op=ALU.add)
                        if not skip_outdma and not skip_drain:
                            nc.sync.dma_start(
                                out[tt * TT:(tt + 1) * TT,
                                    hf * HFW:(hf + 1) * HFW], acc[:])

                # ---- entropy reduction ----
                if not skip_gate:
                    er = gpool.tile([128, 1], f32, tag="er")
                    nc.vector.tensor_reduce(er[:], entc[:], axis=AX.X, op=ALU.add)
                    ep = zpool.tile([1, 1], f32, tag=ztag)
                    nc.tensor.matmul(ep[:], er[:], ones_col[:], start=True, stop=True)
                    esb = gpool.tile([1, 1], f32, tag="esb")
                    nc.vector.tensor_copy(esb[:], ep[:])
                    nc.sync.dma_start(ent[:], esb[:])

            if n_repeat > 1:
                with tc.For_i(0, n_repeat, 1):
                    body()
            else:
                body()

    nc.compile()
    return nc


def _round_fp22(x):
    """Round fp32 to fp22 (13 mantissa bits) — matches PE float32r ingest."""
    v = x.view(np.uint32)
    v = (v + np.uint32(0x200)) & np.uint32(0xFFFFFC00)
    return v.view(np.float32)


def _prep_shared(gate_W, gate_b, exp_W, exp_b):
    """Host-side layout prep shared across cores."""
    # wx[hf, e, p, dt, f] = exp_W[e, hf*512+f, dt*128+p], pre-rounded to fp22
    wx = _round_fp22(np.ascontiguousarray(
        exp_W.reshape(E, NHF, HFW, 8, 128).transpose(1, 0, 4, 3, 2)))
    # gw[p, dt, e] = gate_W[e, dt*128+p]
    gwl = np.ascontiguousarray(gate_W.reshape(E, 9, 128).transpose(2, 1, 0))
    gbl = np.ascontiguousarray(gate_b.reshape(1, E))
    ebl = _round_fp22(np.ascontiguousarray(exp_b))
    return wx, gwl, gbl, ebl


def _prep_core(h_b, a_b, n_ttiles=NT_FULL):
    # hx[tt, p, dt, f] = h[tt*128+f, dt*128+p]
    ntok = n_ttiles * TT
    hx = np.ascontiguousarray(
        h_b[:ntok].reshape(n_ttiles, TT, 8, 128).transpose(0, 3, 2, 1))
    avl = np.ascontiguousarray(a_b.reshape(128, 1))
    return hx, avl


def kernel(h, a, gate_W, gate_b, exp_W, exp_b):
    from concourse import bass_utils

    h = np.asarray(h, dtype=np.float32)
    a = np.asarray(a, dtype=np.float32)
    gate_W = np.asarray(gate_W, dtype=np.float32)
    gate_b = np.asarray(gate_b, dtype=np.float32)
    exp_W = np.asarray(exp_W, dtype=np.float32)
    exp_b = np.asarray(exp_b, dtype=np.float32)

    key = (NT_FULL, 1)
    if key not in _BUILD_CACHE:
        _BUILD_CACHE[key] = _build(*key)
    nc = _BUILD_CACHE[key]

    wx, gwl, gbl, ebl = _prep_shared(gate_W, gate_b, exp_W, exp_b)
    in_maps = []
    for b in range(N_CORES):
        hx, avl = _prep_core(h[b], a[b])
        in_maps.append({"hx": hx, "wx": wx, "gw": gwl, "av": avl,
                        "gb": gbl, "eb": ebl})

    res = bass_utils.run_bass_kernel_spmd(nc, in_maps, core_ids=list(range(N_CORES)))

    out = np.stack([res.results[b]["out"] for b in range(N_CORES)], axis=0)
    ent_sum = np.float64(0.0)
    for b in range(N_CORES):
        ent_sum += np.float64(res.results[b]["ent"][0, 0])
    entropy_loss = np.float32(-(ent_sum / (B * S)))
    stability_loss = np.float32(0.0)
    return out, entropy_loss, stability_loss
